# revision 12
# baseline (speedup 1.0000x reference)
"""MAMGCN submodule kernel for Trainium2, 8-core (batch, column-half) parallel.

Problem (per reference):
  B=16, N=1024, F=64, T=12, K=3, F_OUT=64
  S = softmax_axis1(Vs @ sigmoid(lhs @ rhs^T + bs))
  out = relu(sum_k (cheb_k * S)^T @ x @ Theta_k)

Sharding: 32 tasks = 16 batches x 2 column-halves, 4 tasks per core.
Core d handles batches 4*(d//2)..4*(d//2)+3 for column half d%2. This keeps
cheb-half, Vs^T and bs-half SBUF-resident (loaded once per core) while x
streams per batch.

The program is SPMD (identical on every core), so the per-core column half
is realized purely in host staging: the source-node (row) dimension of
x/bs/Vs/cheb is permuted per core so that the nodes of "my half" always sit
in rows 0..511. The contraction over source nodes is permutation-invariant,
and destination columns are host-sliced per core, so results are exact.

Host-side staging (layout only, no math): x pre-transposed to (B, N, T, F)
bf16; Vs pre-transposed; cheb/bs/Theta cast to bf16; Theta pre-packed as the
(128, K, 128) block-diagonal used by the PE.

Per-core layouts (r = source node on partitions, c = destination column):
  P = sigmoid(lhsT^T rhsT + bs)      [r_p, 8, 512] bf16
  E = exp(VsT^T P)                   [r_p, 8, 512] bf16
  A_k = cheb_k * E                   [r_p, K, 8, 512] bf16
  z'_q = xp_chunk^T A (accum over r) [tf_p, 512] psum, per (t,f)-128-chunk
  o' = sum_k thbd_k^T zs_k           [(t,o)_p, 512] psum
  out = relu(recip[c] * transpose(o')) with softmax denominator folded in
  as a per-partition scalar in the final DVE copy.
All matmuls run bf16 moving operands (full PE rate) except the tiny
feature-space ones which use f32r bitcasts.
"""
import numpy as np
import ml_dtypes

import concourse.bass as bass
import concourse.mybir as mybir
import concourse.tile as tile
from concourse import bacc
from concourse.bass_utils import run_bass_kernel_spmd  # noqa: F401 (contract)

F32 = mybir.dt.float32
F32R = mybir.dt.float32r
BF16 = mybir.dt.bfloat16
AL = mybir.AluOpType
AF = mybir.ActivationFunctionType
AX = mybir.AxisListType

B = 16
N = 1024
F = 64
T = 12
K = 3
FO = 64
NT = N // 128           # 8 r-tiles
HALF = 512              # columns per task
CT = HALF // 128        # 4 c-subtiles
TQ = (T * F) // 128     # 6 (t,f)-chunks of 128
B_PER_CORE = 4          # 4 (batch, half) tasks per core
N_CORES = 8
BF16_NP = ml_dtypes.bfloat16


def _emit_task(nc, tc, pools, cst, bi, xp_d, out_d):
    """One (batch, half) task; the half is implicit in this core's data
    (rows are permuted so my half's nodes are row-tiles 0..3)."""
    (stream, xpp, pe_pool, res_pool, psA, psD, psZ, dram_pool) = pools

    # ---- Stage A: load x' and build attention row/col features ----
    xp = xpp.tile([128, NT, T, F], BF16, tag="xp")
    nc.sync.dma_start(
        out=xp[:],
        in_=xp_d.ap()[bi].rearrange("(mi p) t f -> p mi t f", p=128))
    xw1T = stream.tile([F, N], F32R, tag="xw1T", bufs=1)
    rhsBT = stream.tile([T, HALF], F32R, tag="rhsBT", bufs=2)
    for mi in range(NT):
        # xw1[n,f] = sum_t x'[n,t,f] * W1[t]  (needed for all source nodes)
        tmp1 = stream.tile([128, T, F], BF16, tag="tmp1", bufs=2)
        nc.vector.tensor_mul(tmp1[:], xp[:, mi], cst["w1rep"][:])
        xw1_t = stream.tile([128, F], F32, tag="xw1t", bufs=2)
        nc.vector.tensor_reduce(out=xw1_t[:],
                                in_=tmp1[:].rearrange("p t f -> p f t"),
                                op=AL.add, axis=AX.X)
        pst64 = psA.tile([F, 128], F32, tag="a")
        nc.tensor.transpose(pst64[:], xw1_t[:], cst["ident"][:])
        nc.vector.tensor_copy(xw1T[:, mi * 128:(mi + 1) * 128], pst64[:])
        if mi < CT:
            # rhsB[c,t] = sum_f W3[f] * x'[c,t,f] — destination cols only,
            # which the row permutation puts in tiles 0..3
            tmp2 = stream.tile([128, T, F], BF16, tag="tmp2", bufs=2)
            nc.vector.tensor_mul(tmp2[:], xp[:, mi], cst["w3rep"][:])
            rhsb_t = stream.tile([128, T], F32, tag="rhsbt", bufs=2)
            nc.vector.tensor_reduce(out=rhsb_t[:], in_=tmp2[:], op=AL.add,
                                    axis=AX.X)
            pst12 = psA.tile([T, 128], F32, tag="a")
            nc.tensor.transpose(pst12[:], rhsb_t[:], cst["ident"][:])
            nc.vector.tensor_copy(rhsBT[:, mi * 128:(mi + 1) * 128], pst12[:])

    # ---- Stage B: lhsT = W2^T @ xw1T  (12, N) ----
    lhsT = stream.tile([T, N], F32R, tag="lhsT", bufs=2)
    for h2 in range(2):
        ps_l = psA.tile([T, 512], F32, tag="a")
        nc.tensor.matmul(ps_l[:], cst["w2"][:],
                         xw1T[:, h2 * 512:(h2 + 1) * 512],
                         start=True, stop=True)
        nc.vector.tensor_copy(lhsT[:, h2 * 512:(h2 + 1) * 512], ps_l[:])

    # ---- Stage C: P = sigmoid(product + bs) for my column half ----
    P = pe_pool.tile([128, NT, HALF], BF16, tag="P")
    for mi in range(NT):
        ps_p = psA.tile([128, HALF], F32, tag="a")
        nc.scalar.copy(ps_p[:], cst["bs"][:, mi])
        nc.tensor.matmul(ps_p[:],
                         lhsT[:, mi * 128:(mi + 1) * 128],
                         rhsBT[:],
                         start=False, stop=True, skip_group_check=True)
        nc.scalar.activation(P[:, mi], ps_p[:], AF.Sigmoid)

    # ---- Stage D: E = exp(VsT^T @ P); colsum via ones-matmul ----
    E = pe_pool.tile([128, NT, HALF], BF16, tag="E", bufs=1)
    ps_d = psD.tile([1, HALF], F32, tag="d", bufs=1)
    for rt in range(NT):
        ps_s = psA.tile([128, HALF], F32, tag="a")
        for mi in range(NT):
            nc.tensor.matmul(ps_s[:],
                             cst["vsT"][:, mi, rt * 128:(rt + 1) * 128],
                             P[:, mi], start=(mi == 0), stop=(mi == NT - 1))
        nc.scalar.activation(E[:, rt], ps_s[:], AF.Exp)
        nc.tensor.matmul(ps_d[:], cst["ones"][:], E[:, rt],
                         start=(rt == 0), stop=(rt == NT - 1))
    recip_row = stream.tile([1, HALF], F32, tag="recip", bufs=2)
    nc.vector.reciprocal(recip_row[:], ps_d[:])
    # scatter recip[c] to partitions: recip_sc[p, ci] = recip[ci*128+p]
    rc_d = dram_pool.tile([HALF], F32, tag="rcd", name="rc_d")
    nc.gpsimd.dma_start(out=rc_d.rearrange("(a b) -> a b", a=1),
                        in_=recip_row[:])
    recip_sc = stream.tile([128, CT], F32, tag="recsc", bufs=2)
    nc.gpsimd.dma_start(out=recip_sc[:],
                        in_=rc_d.rearrange("(c p) -> p c", p=128))

    # ---- Stage E: A_k = cheb_k * E ----
    A = pe_pool.tile([128, K, NT, HALF], BF16, tag="A")
    for rt in range(NT):
        for k in range(K):
            nc.vector.tensor_mul(A[:, k, rt], cst["cheb"][:, k, rt], E[:, rt])

    # ---- Stage F: z' chunks; Theta block-diag; transpose out ----
    res_tiles = []
    for ci in range(CT):
        res_c = res_pool.tile([128, FO, T], F32, tag=f"res{ci}")
        res_tiles.append(res_c)
    for q in range(TQ):
        ps_z = psZ.tile([128, K, HALF], F32, tag="z")
        for k in range(K):
            for rt in range(NT):
                nc.tensor.matmul(ps_z[:, k], xp[:, rt, 2 * q:2 * q + 2, :],
                                 A[:, k, rt],
                                 start=(rt == 0), stop=(rt == NT - 1))
        ps_o = psA.tile([128, HALF], F32, tag="a")
        for k in range(K):
            zs = stream.tile([128, HALF], BF16, tag="zs")
            nc.scalar.copy(zs[:], ps_z[:, k])
            nc.tensor.matmul(ps_o[:], cst["thbd"][:, k], zs[:],
                             start=(k == 0), stop=(k == K - 1))
        os_t = stream.tile([128, HALF], F32, tag="os", bufs=2)
        nc.vector.tensor_copy(os_t[:], ps_o[:])
        for ci in range(CT):
            ps_tr = psA.tile([128, 128], F32, tag="a")
            nc.tensor.transpose(ps_tr[:], os_t[:, ci * 128:(ci + 1) * 128],
                                cst["ident"][:])
            nc.vector.tensor_scalar(
                out=res_tiles[ci][:, :, 2 * q:2 * q + 2],
                in0=ps_tr[:].rearrange("p (dt o) -> p o dt", o=FO),
                scalar1=recip_sc[:, ci:ci + 1],
                scalar2=0.0,
                op0=AL.mult,
                op1=AL.max,
            )
    for ci in range(CT):
        nc.sync.dma_start(out=out_d.ap()[bi, ci * 128:(ci + 1) * 128],
                          in_=res_tiles[ci][:])


def build_nc(repeat=1):
    nc = bacc.Bacc("TRN2", target_bir_lowering=False, debug=False,
                   num_devices=N_CORES)
    xp_d = nc.dram_tensor("xp", [B_PER_CORE, N, T, F], BF16,
                          kind="ExternalInput")
    vsT_d = nc.dram_tensor("vsT", [N, N], BF16, kind="ExternalInput")
    cheb_d = nc.dram_tensor("chebh", [K, N, HALF], BF16, kind="ExternalInput")
    bs_d = nc.dram_tensor("bsh", [N, HALF], BF16, kind="ExternalInput")
    w1rep_d = nc.dram_tensor("w1rep", [128, T, F], BF16, kind="ExternalInput")
    w3rep_d = nc.dram_tensor("w3rep", [128, T, F], BF16, kind="ExternalInput")
    w2_d = nc.dram_tensor("w2", [F, T], F32, kind="ExternalInput")
    thbd_d = nc.dram_tensor("thbd", [128, K, 128], BF16, kind="ExternalInput")
    ident_d = nc.dram_tensor("ident", [128, 128], F32, kind="ExternalInput")
    out_d = nc.dram_tensor("out", [B_PER_CORE, HALF, FO, T], F32,
                           kind="ExternalOutput")

    with tile.TileContext(nc) as tc:
        with (
            tc.tile_pool(name="consts", bufs=1) as consts,
            tc.tile_pool(name="stream", bufs=3) as stream,
            tc.tile_pool(name="xpp", bufs=2) as xpp,
            tc.tile_pool(name="pe", bufs=2) as pe_pool,
            tc.tile_pool(name="res", bufs=2) as res_pool,
            tc.tile_pool(name="dram", bufs=2, space="DRAM") as dram_pool,
            tc.tile_pool(name="psA", bufs=3, space="PSUM") as psA,
            tc.tile_pool(name="psD", bufs=1, space="PSUM") as psD,
            tc.tile_pool(name="psZ", bufs=1, space="PSUM") as psZ,
        ):
            cst = {}
            ident = consts.tile([128, 128], F32)
            nc.sync.dma_start(out=ident[:], in_=ident_d.ap())
            cst["ident"] = ident
            ones = consts.tile([128, 1], BF16)
            nc.vector.memset(ones[:], 1.0)
            cst["ones"] = ones
            w1rep = consts.tile([128, T, F], BF16)
            nc.sync.dma_start(out=w1rep[:], in_=w1rep_d.ap())
            cst["w1rep"] = w1rep
            w3rep = consts.tile([128, T, F], BF16)
            nc.sync.dma_start(out=w3rep[:], in_=w3rep_d.ap())
            cst["w3rep"] = w3rep
            w2f = consts.tile([F, T], F32)
            nc.sync.dma_start(out=w2f[:], in_=w2_d.ap())
            w2 = consts.tile([F, T], F32R)
            nc.vector.tensor_copy(w2[:], w2f[:])
            cst["w2"] = w2
            thbd = consts.tile([128, K, 128], BF16)
            nc.sync.dma_start(out=thbd[:], in_=thbd_d.ap())
            cst["thbd"] = thbd
            bs_sb = consts.tile([128, NT, HALF], BF16)
            nc.sync.dma_start(out=bs_sb[:],
                              in_=bs_d.ap().rearrange("(mi p) c -> p mi c",
                                                      p=128))
            cst["bs"] = bs_sb
            vsT = consts.tile([128, NT, N], BF16)
            nc.scalar.dma_start(out=vsT[:],
                                in_=vsT_d.ap().rearrange("(mi p) r -> p mi r",
                                                         p=128))
            cst["vsT"] = vsT
            cheb_sb = consts.tile([128, K, NT, HALF], BF16)
            nc.gpsimd.dma_start(
                out=cheb_sb[:],
                in_=cheb_d.ap().rearrange("k (rt p) c -> p k rt c", p=128))
            cst["cheb"] = cheb_sb

            pools = (stream, xpp, pe_pool, res_pool, psA, psD, psZ, dram_pool)
            for _ in range(repeat):
                for bi in range(B_PER_CORE):
                    _emit_task(nc, tc, pools, cst, bi, xp_d, out_d)
    nc.compile()
    return nc


_RUNNER_CACHE = {}


def _make_runner(repeat=1):
    """Build the Bass program once and wrap it in a persistent jitted
    shard_map executable so repeat calls skip recompile/reload."""
    import jax
    from jax.sharding import Mesh, PartitionSpec
    from jax.experimental.shard_map import shard_map
    from concourse import bass2jax, mybir as _mybir

    nc = build_nc(repeat)
    bass2jax.install_neuronx_cc_hook()

    part_name = nc.partition_id_tensor.name if nc.partition_id_tensor else None
    in_names = []
    out_names = []
    out_avals = []
    zero_outs = []
    for alloc in nc.m.functions[0].allocations:
        if not isinstance(_mybir.MemoryLocationSet, type) or not isinstance(
                alloc, _mybir.MemoryLocationSet):
            continue
        name = alloc.memorylocations[0].name
        if alloc.kind == "ExternalInput":
            if name != part_name:
                in_names.append(name)
        elif alloc.kind == "ExternalOutput":
            out_names.append(name)
            shape = tuple(alloc.tensor_shape)
            dtype = _mybir.dt.np(alloc.dtype)
            out_avals.append(jax.core.ShapedArray(shape, dtype))
            zero_outs.append(np.zeros(shape, dtype))
    all_names = in_names + out_names
    if part_name is not None:
        all_names = all_names + [part_name]

    def _body(*args):
        operands = list(args)
        if part_name is not None:
            operands.append(bass2jax.partition_id_tensor())
        outs = bass2jax._bass_exec_p.bind(
            *operands,
            out_avals=tuple(out_avals),
            in_names=tuple(all_names),
            out_names=tuple(out_names),
            lowering_input_output_aliases=(),
            sim_require_finite=False,
            sim_require_nnan=False,
            nc=nc,
        )
        return tuple(outs)

    devices = jax.devices()[:N_CORES]
    mesh = Mesh(np.asarray(devices), ("core",))
    percore = {"xp", "chebh", "bsh", "vsT"}
    in_specs = tuple(
        PartitionSpec("core") if name in percore else PartitionSpec()
        for name in in_names
    ) + (PartitionSpec("core"),) * len(out_names)
    out_specs = (PartitionSpec("core"),) * len(out_names)
    sharded = jax.jit(
        shard_map(_body, mesh=mesh, in_specs=in_specs, out_specs=out_specs,
                  check_rep=False),
        keep_unused=True,
    )
    return nc, sharded, in_names, out_names, zero_outs, mesh


def _get_runner(repeat=1):
    if repeat not in _RUNNER_CACHE:
        _RUNNER_CACHE[repeat] = _make_runner(repeat)
    return _RUNNER_CACHE[repeat]


def _stage_inputs(x, W1, W2, W3, bs, Vs, cheb, Theta):
    """Host-side layout staging for the 8 cores (no math, only
    transpose/cast/slice/permute)."""
    x = np.asarray(x, dtype=np.float32)
    # x' = (B, N, T, F) bf16; core d gets batches of group d//2, with the
    # node (row) axis rolled so its column half h=d%2 sits in rows 0..511
    xp = np.ascontiguousarray(x.transpose(0, 1, 3, 2)).astype(BF16_NP)
    xp_sw = np.ascontiguousarray(np.roll(xp, HALF, axis=1))  # h=1 perm
    xp_dev = np.empty((N_CORES, 4, N, T, F), dtype=BF16_NP)
    for d in range(N_CORES):
        g, h = d // 2, d % 2
        xp_dev[d] = (xp_sw if h else xp)[4 * g:4 * g + 4]
    xp_dev = xp_dev.reshape(N_CORES * 4, N, T, F)

    cheb_bf = np.asarray(cheb, dtype=np.float32).astype(BF16_NP)
    cheb_dev = np.empty((N_CORES, K, N, HALF), dtype=BF16_NP)
    bs2 = np.asarray(bs, dtype=np.float32).reshape(N, N).astype(BF16_NP)
    bs_dev = np.empty((N_CORES, N, HALF), dtype=BF16_NP)
    vsT_full = np.ascontiguousarray(
        np.asarray(Vs, dtype=np.float32).T).astype(BF16_NP)
    vsT_dev = np.empty((N_CORES, N, N), dtype=BF16_NP)
    for d in range(N_CORES):
        h = d % 2
        lo = h * HALF
        # row permutation (sigma): roll by HALF when h=1
        cheb_dev[d] = np.roll(cheb_bf, HALF, axis=1)[:, :, lo:lo + HALF] \
            if h else cheb_bf[:, :, lo:lo + HALF]
        bs_dev[d] = np.roll(bs2, HALF, axis=0)[:, lo:lo + HALF] \
            if h else bs2[:, lo:lo + HALF]
        # vsT[m, r]: both m (sig rows) and r (E rows) follow sigma
        vsT_dev[d] = np.roll(np.roll(vsT_full, HALF, axis=0), HALF, axis=1) \
            if h else vsT_full
    cheb_dev = cheb_dev.reshape(N_CORES * K, N, HALF)
    vsT = vsT_dev.reshape(N_CORES * N, N)

    w1 = np.asarray(W1, dtype=np.float32)
    w3 = np.asarray(W3, dtype=np.float32)
    w1rep = np.ascontiguousarray(
        np.broadcast_to(w1[None, :, None], (128, T, F))).astype(BF16_NP)
    w3rep = np.ascontiguousarray(
        np.broadcast_to(w3[None, None, :], (128, T, F))).astype(BF16_NP)
    w2 = np.ascontiguousarray(np.asarray(W2, dtype=np.float32))

    th = np.asarray(Theta, dtype=np.float32)
    thbd = np.zeros((128, K, 128), dtype=BF16_NP)
    for t2 in range(2):
        thbd[t2 * 64:(t2 + 1) * 64, :, t2 * 64:(t2 + 1) * 64] = (
            th.transpose(1, 0, 2).astype(BF16_NP))
    ident = np.eye(128, dtype=np.float32)

    return {
        "xp": xp_dev, "vsT": vsT, "chebh": cheb_dev, "bsh": bs_dev,
        "w1rep": w1rep, "w3rep": w3rep, "w2": w2, "thbd": thbd,
        "ident": ident,
    }


def _staged_ops(staged, in_names, zero_outs):
    ops = [staged[name] for name in in_names]
    for z in zero_outs:
        ops.append(np.zeros((N_CORES * z.shape[0], *z.shape[1:]), z.dtype))
    return ops


def _gather_out(out_arr):
    # per-core [4, 512, 64, 12]; core d=(2g+h): batches 4g+i, cols half h
    arr = np.asarray(out_arr).reshape(4, 2, 4, HALF, FO, T)
    arr = arr.transpose(0, 2, 1, 3, 4, 5).reshape(B, N, FO, T)
    return np.ascontiguousarray(arr)


def kernel(x, W1, W2, W3, bs, Vs, cheb, Theta, repeat=1):
    staged = _stage_inputs(x, W1, W2, W3, bs, Vs, cheb, Theta)
    nc, sharded, in_names, out_names, zero_outs, mesh = _get_runner(repeat)
    ops = _staged_ops(staged, in_names, zero_outs)
    out_arrs = sharded(*ops)
    return _gather_out(out_arrs[out_names.index("out")])


def _bench_setup(inputs, repeat):
    import jax
    from jax.sharding import NamedSharding, PartitionSpec
    staged = _stage_inputs(**inputs)
    nc, sharded, in_names, out_names, zero_outs, mesh = _get_runner(repeat)
    ops = _staged_ops(staged, in_names, zero_outs)
    percore = {"xp", "chebh", "bsh", "vsT"}
    sh_core = NamedSharding(mesh, PartitionSpec("core"))
    sh_rep = NamedSharding(mesh, PartitionSpec())
    shardings = [sh_core if name in percore else sh_rep for name in in_names]
    shardings += [sh_core] * len(zero_outs)
    dev_ops = [jax.device_put(o, s_) for o, s_ in zip(ops, shardings)]
    jax.block_until_ready(sharded(*dev_ops))
    return sharded, dev_ops


def bench_pair(inputs, rep_a=1, rep_b=9, iters=20):
    """Interleaved device-resident timing of two repeat variants.
    Returns (best_a, best_b) seconds."""
    import time as _time
    import jax
    sh_a, ops_a = _bench_setup(inputs, rep_a)
    sh_b, ops_b = _bench_setup(inputs, rep_b)
    best_a = best_b = float("inf")
    for _ in range(iters):
        t0 = _time.time()
        jax.block_until_ready(sh_a(*ops_a))
        best_a = min(best_a, _time.time() - t0)
        t0 = _time.time()
        jax.block_until_ready(sh_b(*ops_b))
        best_b = min(best_b, _time.time() - t0)
    return best_a, best_b


# revision 22
# speedup vs baseline: 16.0798x; 16.0798x over previous
"""MAMGCN submodule kernel for Trainium2, 8-core (batch, column-half) parallel.

Problem (per reference):
  B=16, N=1024, F=64, T=12, K=3, F_OUT=64
  S = softmax_axis1(Vs @ sigmoid(lhs @ rhs^T + bs))
  out = relu(sum_k (cheb_k * S)^T @ x @ Theta_k)

Sharding: 32 tasks = 16 batches x 2 column-halves, 4 tasks per core.
Core d handles batches 4*(d//2)..4*(d//2)+3 for column half d%2. This keeps
cheb-half, Vs^T and bs-half SBUF-resident (loaded once per core) while x
streams per batch.

The program is SPMD (identical on every core), so the per-core column half
is realized purely in host staging: the source-node (row) dimension of
x/bs/Vs/cheb is permuted per core so that the nodes of "my half" always sit
in rows 0..511. The contraction over source nodes is permutation-invariant,
and destination columns are host-sliced per core, so results are exact.

Host-side staging (layout only, no math): x pre-transposed to (B, N, T, F)
bf16; Vs pre-transposed; cheb/bs/Theta cast to bf16; Theta pre-packed as the
(128, K, 128) block-diagonal used by the PE.

Per-core layouts (r = source node on partitions, c = destination column):
  P = sigmoid(lhsT^T rhsT + bs)      [r_p, 8, 512] bf16
  E = exp(VsT^T P)                   [r_p, 8, 512] bf16
  A_k = cheb_k * E                   [r_p, K, 8, 512] bf16
  z'_q = xp_chunk^T A (accum over r) [tf_p, 512] psum, per (t,f)-128-chunk
  o' = sum_k thbd_k^T zs_k           [(t,o)_p, 512] psum
  out = relu(recip[c] * transpose(o')) with softmax denominator folded in
  as a per-partition scalar in the final DVE copy.
All matmuls run bf16 moving operands (full PE rate) except the tiny
feature-space ones which use f32r bitcasts.
"""
import numpy as np
import ml_dtypes

import concourse.bass as bass
import concourse.mybir as mybir
import concourse.tile as tile
from concourse import bacc
from concourse.bass_utils import run_bass_kernel_spmd  # noqa: F401 (contract)

F32 = mybir.dt.float32
F32R = mybir.dt.float32r
BF16 = mybir.dt.bfloat16
AL = mybir.AluOpType
AF = mybir.ActivationFunctionType
AX = mybir.AxisListType

B = 16
N = 1024
F = 64
T = 12
K = 3
FO = 64
NT = N // 128           # 8 r-tiles
HALF = 512              # columns per task
CT = HALF // 128        # 4 c-subtiles
TQ = (T * F) // 128     # 6 (t,f)-chunks of 128
B_PER_CORE = 4          # 4 (batch, half) tasks per core
N_CORES = 8
BF16_NP = ml_dtypes.bfloat16


def _emit_stageABC(nc, pools, cst, bi, xp_d, first=False):
    """Stages A-C of one (batch, half) task: features, product, sigmoid.
    The half is implicit in this core's data (rows are permuted so my
    half's nodes are row-tiles 0..3)."""
    (stream, xpp, pe_pool, res_pool, psA, psD, psZ, psT, dram_pool) = pools

    # ---- Stage A: load x' and build attention row/col features ----
    xp = xpp.tile([128, NT, T, F], BF16, tag="xp")
    nc.sync.dma_start(
        out=xp[:],
        in_=xp_d.ap()[bi].rearrange("(mi p) t f -> p mi t f", p=128))
    xw1T = stream.tile([F, N], F32R, tag="xw1T", bufs=1)
    rhsBT = stream.tile([T, HALF], F32R, tag="rhsBT", bufs=1)
    for mi in range(NT):
        # xw1[n,f] = sum_t x'[n,t,f] * W1[t]  (needed for all source nodes)
        tmp1 = stream.tile([128, T, F], F32, tag="tmp1", bufs=2)
        mul_eng = nc.vector if first else nc.gpsimd
        mul_eng.tensor_mul(tmp1[:], xp[:, mi], cst["w1rep"][:])
        xw1_t = stream.tile([128, F], F32, tag="xw1t", bufs=2)
        nc.vector.tensor_reduce(out=xw1_t[:],
                                in_=tmp1[:].rearrange("p t f -> p f t"),
                                op=AL.add, axis=AX.X)
        pst64 = psA.tile([F, 128], F32, tag="a")
        nc.tensor.transpose(pst64[:], xw1_t[:], cst["ident"][:])
        nc.vector.tensor_copy(xw1T[:, mi * 128:(mi + 1) * 128], pst64[:])
        if mi < CT:
            # rhsB[c,t] = sum_f W3[f] * x'[c,t,f] — destination cols only,
            # which the row permutation puts in tiles 0..3
            tmp2 = stream.tile([128, T, F], F32, tag="tmp2", bufs=2)
            mul_eng.tensor_mul(tmp2[:], xp[:, mi], cst["w3rep"][:])
            rhsb_t = stream.tile([128, T], F32, tag="rhsbt", bufs=2)
            nc.vector.tensor_reduce(out=rhsb_t[:], in_=tmp2[:], op=AL.add,
                                    axis=AX.X)
            pst12 = psA.tile([T, 128], F32, tag="a")
            nc.tensor.transpose(pst12[:], rhsb_t[:], cst["ident"][:])
            nc.vector.tensor_copy(rhsBT[:, mi * 128:(mi + 1) * 128], pst12[:])

    # ---- Stage B: lhsT = W2^T @ xw1T  (12, N) ----
    lhsT = stream.tile([T, N], F32R, tag="lhsT", bufs=1)
    for h2 in range(2):
        ps_l = psA.tile([T, 512], F32, tag="a")
        nc.tensor.matmul(ps_l[:], cst["w2"][:],
                         xw1T[:, h2 * 512:(h2 + 1) * 512],
                         start=True, stop=True)
        nc.vector.tensor_copy(lhsT[:, h2 * 512:(h2 + 1) * 512], ps_l[:])

    # ---- Stage C: P = sigmoid(product + bs) for my column half ----
    P = pe_pool.tile([128, NT, HALF], BF16, tag="P")
    for mi in range(NT):
        ps_p = psA.tile([128, HALF], F32, tag="a")
        nc.tensor.matmul(ps_p[:],
                         lhsT[:, mi * 128:(mi + 1) * 128],
                         rhsBT[:],
                         start=True, stop=True)
        nc.vector.tensor_add(ps_p[:], ps_p[:], cst["bs"][:, mi])
        nc.scalar.activation(P[:, mi], ps_p[:], AF.Sigmoid)
    return {"bi": bi, "xp": xp, "P": P}


def _emit_stageD(nc, pools, cst, st):
    """Stage D: E = exp(VsT^T @ P); softmax denominators -> recip_sc."""
    (stream, xpp, pe_pool, res_pool, psA, psD, psZ, psT, dram_pool) = pools
    P = st["P"]
    E = pe_pool.tile([128, NT, HALF], BF16, tag="E", bufs=1)
    ps_d = psD.tile([1, HALF], F32, tag="d", bufs=1)
    for rt in range(NT):
        ps_s = psA.tile([128, HALF], F32, tag="a")
        for mi in range(NT):
            nc.tensor.matmul(ps_s[:],
                             cst["vsT"][:, mi, rt * 128:(rt + 1) * 128],
                             P[:, mi], start=(mi == 0), stop=(mi == NT - 1))
        nc.scalar.activation(E[:, rt], ps_s[:], AF.Exp)
        nc.tensor.matmul(ps_d[:], cst["ones"][:], E[:, rt],
                         start=(rt == 0), stop=(rt == NT - 1))
    recip_row = stream.tile([1, HALF], F32, tag="recip", bufs=1)
    nc.vector.reciprocal(recip_row[:], ps_d[:])
    # scatter recip[c] to partitions: recip_sc[p, ci] = recip[ci*128+p]
    rc_d = dram_pool.tile([HALF], F32, tag="rcd", name="rc_d")
    nc.gpsimd.dma_start(out=rc_d.rearrange("(a b) -> a b", a=1),
                        in_=recip_row[:])
    recip_sc = stream.tile([128, CT], F32, tag="recsc", bufs=2)
    nc.gpsimd.dma_start(out=recip_sc[:],
                        in_=rc_d.rearrange("(c p) -> p c", p=128))
    st["E"] = E
    st["recip_sc"] = recip_sc


def _emit_stageE(nc, pools, cst, st):
    """Stage E: A_k = cheb_k * E (k-major so z can start after 8 mults)."""
    (stream, xpp, pe_pool, res_pool, psA, psD, psZ, psT, dram_pool) = pools
    E = st["E"]
    A = pe_pool.tile([128, K, NT, HALF], BF16, tag="A")
    for k in range(K):
        for rt in range(NT):
            nc.vector.tensor_mul(A[:, k, rt], cst["cheb"][:, k, rt], E[:, rt])
    st["A"] = A
    st["res"] = [res_pool.tile([128, FO, T], F32, tag=f"res{ci}",
                               name=f"res{ci}")
                 for ci in range(CT)]


def _emit_stageF(nc, pools, cst, st, out_d, q_lo, q_hi):
    """Stage F for q in [q_lo, q_hi): z' chunks, Theta, transpose out.
    Emits the output DMAs after the last chunk."""
    (stream, xpp, pe_pool, res_pool, psA, psD, psZ, psT, dram_pool) = pools
    xp, A, recip_sc, res_tiles = st["xp"], st["A"], st["recip_sc"], st["res"]
    for q in range(q_lo, q_hi):
        ps_z = psZ.tile([128, K, HALF], F32, tag="z")
        for k in range(K):
            for rt in range(NT):
                nc.tensor.matmul(ps_z[:, k], xp[:, rt, 2 * q:2 * q + 2, :],
                                 A[:, k, rt],
                                 start=(rt == 0), stop=(rt == NT - 1))
        ps_o = psA.tile([128, HALF], F32, tag="a")
        for k in range(K):
            zs = stream.tile([128, HALF], BF16, tag="zs")
            nc.scalar.copy(zs[:], ps_z[:, k])
            nc.tensor.matmul(ps_o[:], cst["thbd"][:, k], zs[:],
                             start=(k == 0), stop=(k == K - 1))
        os_t = stream.tile([128, HALF], F32, tag="os", bufs=2)
        nc.scalar.copy(os_t[:], ps_o[:])
        for ci in range(CT):
            ps_tr = psT.tile([128, 128], F32, tag="tr")
            nc.tensor.transpose(ps_tr[:], os_t[:, ci * 128:(ci + 1) * 128],
                                cst["ident"][:])
            nc.scalar.activation(
                res_tiles[ci][:, :, 2 * q:2 * q + 2],
                ps_tr[:].rearrange("p (dt o) -> p o dt", o=FO),
                AF.Relu, scale=recip_sc[:, ci:ci + 1])
    if q_hi == TQ:
        for ci in range(CT):
            nc.sync.dma_start(
                out=out_d.ap()[st["bi"], ci * 128:(ci + 1) * 128],
                in_=res_tiles[ci][:])


def _emit_pipeline(nc, pools, cst, xp_d, out_d, repeat):
    """Software-pipelined emission: task t-1's PE-heavy stage F is
    interleaved between task t's stages C and D so the in-order PE queue
    has fill work while Act/DVE produce P, E and A for task t."""
    prev = None
    for _ in range(repeat):
        for bi in range(B_PER_CORE):
            if prev is not None:
                _emit_stageF(nc, pools, cst, prev, out_d, 0, 3)
            st = _emit_stageABC(nc, pools, cst, bi, xp_d,
                                    first=(prev is None))
            if prev is not None:
                _emit_stageF(nc, pools, cst, prev, out_d, 3, TQ)
            _emit_stageD(nc, pools, cst, st)
            _emit_stageE(nc, pools, cst, st)
            prev = st
    _emit_stageF(nc, pools, cst, prev, out_d, 0, 3)
    _emit_stageF(nc, pools, cst, prev, out_d, 3, TQ)


def build_nc(repeat=1):
    nc = bacc.Bacc("TRN2", target_bir_lowering=False, debug=False,
                   num_devices=N_CORES)
    xp_d = nc.dram_tensor("xp", [B_PER_CORE, N, T, F], BF16,
                          kind="ExternalInput")
    vsT_d = nc.dram_tensor("vsT", [N, N], BF16, kind="ExternalInput")
    cheb_d = nc.dram_tensor("chebh", [K, N, HALF], BF16, kind="ExternalInput")
    bs_d = nc.dram_tensor("bsh", [N, HALF], BF16, kind="ExternalInput")
    w1rep_d = nc.dram_tensor("w1rep", [128, T, F], BF16, kind="ExternalInput")
    w3rep_d = nc.dram_tensor("w3rep", [128, T, F], BF16, kind="ExternalInput")
    w2_d = nc.dram_tensor("w2", [F, T], F32, kind="ExternalInput")
    thbd_d = nc.dram_tensor("thbd", [128, K, 128], BF16, kind="ExternalInput")
    ident_d = nc.dram_tensor("ident", [128, 128], F32, kind="ExternalInput")
    out_d = nc.dram_tensor("out", [B_PER_CORE, HALF, FO, T], F32,
                           kind="ExternalOutput")

    with tile.TileContext(nc) as tc:
        with (
            tc.tile_pool(name="consts", bufs=1) as consts,
            tc.tile_pool(name="stream", bufs=3) as stream,
            tc.tile_pool(name="xpp", bufs=2) as xpp,
            tc.tile_pool(name="pe", bufs=2) as pe_pool,
            tc.tile_pool(name="res", bufs=2) as res_pool,
            tc.tile_pool(name="dram", bufs=2, space="DRAM") as dram_pool,
            tc.tile_pool(name="psA", bufs=2, space="PSUM") as psA,
            tc.tile_pool(name="psT", bufs=2, space="PSUM") as psT,
            tc.tile_pool(name="psD", bufs=1, space="PSUM") as psD,
            tc.tile_pool(name="psZ", bufs=1, space="PSUM") as psZ,
        ):
            cst = {}
            ident = consts.tile([128, 128], F32)
            nc.sync.dma_start(out=ident[:], in_=ident_d.ap())
            cst["ident"] = ident
            ones = consts.tile([128, 1], BF16)
            nc.vector.memset(ones[:], 1.0)
            cst["ones"] = ones
            w1rep = consts.tile([128, T, F], BF16)
            nc.sync.dma_start(out=w1rep[:], in_=w1rep_d.ap())
            cst["w1rep"] = w1rep
            w3rep = consts.tile([128, T, F], BF16)
            nc.sync.dma_start(out=w3rep[:], in_=w3rep_d.ap())
            cst["w3rep"] = w3rep
            w2f = consts.tile([F, T], F32)
            nc.sync.dma_start(out=w2f[:], in_=w2_d.ap())
            w2 = consts.tile([F, T], F32R)
            nc.vector.tensor_copy(w2[:], w2f[:])
            cst["w2"] = w2
            thbd = consts.tile([128, K, 128], BF16)
            nc.sync.dma_start(out=thbd[:], in_=thbd_d.ap())
            cst["thbd"] = thbd
            bs_sb = consts.tile([128, NT, HALF], BF16)
            nc.scalar.dma_start(out=bs_sb[:],
                                in_=bs_d.ap().rearrange("(mi p) c -> p mi c",
                                                        p=128))
            cst["bs"] = bs_sb
            vsT = consts.tile([128, NT, N], BF16)
            nc.scalar.dma_start(out=vsT[:],
                                in_=vsT_d.ap().rearrange("(mi p) r -> p mi r",
                                                         p=128))
            cst["vsT"] = vsT
            cheb_sb = consts.tile([128, K, NT, HALF], BF16)
            nc.gpsimd.dma_start(
                out=cheb_sb[:],
                in_=cheb_d.ap().rearrange("k (rt p) c -> p k rt c", p=128))
            cst["cheb"] = cheb_sb

            pools = (stream, xpp, pe_pool, res_pool, psA, psD, psZ, psT,
                     dram_pool)
            _emit_pipeline(nc, pools, cst, xp_d, out_d, repeat)
    nc.compile()
    return nc


_RUNNER_CACHE = {}


def _make_runner(repeat=1):
    """Build the Bass program once and wrap it in a persistent jitted
    shard_map executable so repeat calls skip recompile/reload."""
    import jax
    from jax.sharding import Mesh, PartitionSpec
    from jax.experimental.shard_map import shard_map
    from concourse import bass2jax, mybir as _mybir

    nc = build_nc(repeat)
    bass2jax.install_neuronx_cc_hook()

    part_name = nc.partition_id_tensor.name if nc.partition_id_tensor else None
    in_names = []
    out_names = []
    out_avals = []
    zero_outs = []
    for alloc in nc.m.functions[0].allocations:
        if not isinstance(_mybir.MemoryLocationSet, type) or not isinstance(
                alloc, _mybir.MemoryLocationSet):
            continue
        name = alloc.memorylocations[0].name
        if alloc.kind == "ExternalInput":
            if name != part_name:
                in_names.append(name)
        elif alloc.kind == "ExternalOutput":
            out_names.append(name)
            shape = tuple(alloc.tensor_shape)
            dtype = _mybir.dt.np(alloc.dtype)
            out_avals.append(jax.core.ShapedArray(shape, dtype))
            zero_outs.append(np.zeros(shape, dtype))
    all_names = in_names + out_names
    if part_name is not None:
        all_names = all_names + [part_name]

    def _body(*args):
        operands = list(args)
        if part_name is not None:
            operands.append(bass2jax.partition_id_tensor())
        outs = bass2jax._bass_exec_p.bind(
            *operands,
            out_avals=tuple(out_avals),
            in_names=tuple(all_names),
            out_names=tuple(out_names),
            lowering_input_output_aliases=(),
            sim_require_finite=False,
            sim_require_nnan=False,
            nc=nc,
        )
        return tuple(outs)

    devices = jax.devices()[:N_CORES]
    mesh = Mesh(np.asarray(devices), ("core",))
    percore = {"xp", "chebh", "bsh", "vsT"}
    in_specs = tuple(
        PartitionSpec("core") if name in percore else PartitionSpec()
        for name in in_names
    ) + (PartitionSpec("core"),) * len(out_names)
    out_specs = (PartitionSpec("core"),) * len(out_names)
    sharded = jax.jit(
        shard_map(_body, mesh=mesh, in_specs=in_specs, out_specs=out_specs,
                  check_rep=False),
        keep_unused=True,
    )
    return nc, sharded, in_names, out_names, zero_outs, mesh


def _get_runner(repeat=1):
    if repeat not in _RUNNER_CACHE:
        _RUNNER_CACHE[repeat] = _make_runner(repeat)
    return _RUNNER_CACHE[repeat]


def _stage_inputs(x, W1, W2, W3, bs, Vs, cheb, Theta):
    """Host-side layout staging for the 8 cores (no math, only
    transpose/cast/slice/permute)."""
    x = np.asarray(x, dtype=np.float32)
    # x' = (B, N, T, F) bf16; core d gets batches of group d//2, with the
    # node (row) axis rolled so its column half h=d%2 sits in rows 0..511
    xp = np.ascontiguousarray(x.transpose(0, 1, 3, 2)).astype(BF16_NP)
    xp_sw = np.ascontiguousarray(np.roll(xp, HALF, axis=1))  # h=1 perm
    xp_dev = np.empty((N_CORES, 4, N, T, F), dtype=BF16_NP)
    for d in range(N_CORES):
        g, h = d // 2, d % 2
        xp_dev[d] = (xp_sw if h else xp)[4 * g:4 * g + 4]
    xp_dev = xp_dev.reshape(N_CORES * 4, N, T, F)

    cheb_bf = np.asarray(cheb, dtype=np.float32).astype(BF16_NP)
    cheb_dev = np.empty((N_CORES, K, N, HALF), dtype=BF16_NP)
    bs2 = np.asarray(bs, dtype=np.float32).reshape(N, N).astype(BF16_NP)
    bs_dev = np.empty((N_CORES, N, HALF), dtype=BF16_NP)
    vsT_full = np.ascontiguousarray(
        np.asarray(Vs, dtype=np.float32).T).astype(BF16_NP)
    vsT_dev = np.empty((N_CORES, N, N), dtype=BF16_NP)
    for d in range(N_CORES):
        h = d % 2
        lo = h * HALF
        # row permutation (sigma): roll by HALF when h=1
        cheb_dev[d] = np.roll(cheb_bf, HALF, axis=1)[:, :, lo:lo + HALF] \
            if h else cheb_bf[:, :, lo:lo + HALF]
        bs_dev[d] = np.roll(bs2, HALF, axis=0)[:, lo:lo + HALF] \
            if h else bs2[:, lo:lo + HALF]
        # vsT[m, r]: both m (sig rows) and r (E rows) follow sigma
        vsT_dev[d] = np.roll(np.roll(vsT_full, HALF, axis=0), HALF, axis=1) \
            if h else vsT_full
    cheb_dev = cheb_dev.reshape(N_CORES * K, N, HALF)
    vsT = vsT_dev.reshape(N_CORES * N, N)

    w1 = np.asarray(W1, dtype=np.float32)
    w3 = np.asarray(W3, dtype=np.float32)
    w1rep = np.ascontiguousarray(
        np.broadcast_to(w1[None, :, None], (128, T, F))).astype(BF16_NP)
    w3rep = np.ascontiguousarray(
        np.broadcast_to(w3[None, None, :], (128, T, F))).astype(BF16_NP)
    w2 = np.ascontiguousarray(np.asarray(W2, dtype=np.float32))

    th = np.asarray(Theta, dtype=np.float32)
    thbd = np.zeros((128, K, 128), dtype=BF16_NP)
    for t2 in range(2):
        thbd[t2 * 64:(t2 + 1) * 64, :, t2 * 64:(t2 + 1) * 64] = (
            th.transpose(1, 0, 2).astype(BF16_NP))
    ident = np.eye(128, dtype=np.float32)

    return {
        "xp": xp_dev, "vsT": vsT, "chebh": cheb_dev, "bsh": bs_dev,
        "w1rep": w1rep, "w3rep": w3rep, "w2": w2, "thbd": thbd,
        "ident": ident,
    }


def _staged_ops(staged, in_names, zero_outs):
    ops = [staged[name] for name in in_names]
    for z in zero_outs:
        ops.append(np.zeros((N_CORES * z.shape[0], *z.shape[1:]), z.dtype))
    return ops


def _gather_out(out_arr):
    # per-core [4, 512, 64, 12]; core d=(2g+h): batches 4g+i, cols half h
    arr = np.asarray(out_arr).reshape(4, 2, 4, HALF, FO, T)
    arr = arr.transpose(0, 2, 1, 3, 4, 5).reshape(B, N, FO, T)
    return np.ascontiguousarray(arr)


def kernel(x, W1, W2, W3, bs, Vs, cheb, Theta, repeat=1):
    staged = _stage_inputs(x, W1, W2, W3, bs, Vs, cheb, Theta)
    nc, sharded, in_names, out_names, zero_outs, mesh = _get_runner(repeat)
    ops = _staged_ops(staged, in_names, zero_outs)
    out_arrs = sharded(*ops)
    return _gather_out(out_arrs[out_names.index("out")])


def _bench_setup(inputs, repeat):
    import jax
    from jax.sharding import NamedSharding, PartitionSpec
    staged = _stage_inputs(**inputs)
    nc, sharded, in_names, out_names, zero_outs, mesh = _get_runner(repeat)
    ops = _staged_ops(staged, in_names, zero_outs)
    percore = {"xp", "chebh", "bsh", "vsT"}
    sh_core = NamedSharding(mesh, PartitionSpec("core"))
    sh_rep = NamedSharding(mesh, PartitionSpec())
    shardings = [sh_core if name in percore else sh_rep for name in in_names]
    shardings += [sh_core] * len(zero_outs)
    dev_ops = [jax.device_put(o, s_) for o, s_ in zip(ops, shardings)]
    jax.block_until_ready(sharded(*dev_ops))
    return sharded, dev_ops


def bench_pair(inputs, rep_a=1, rep_b=9, iters=20):
    """Interleaved device-resident timing of two repeat variants.
    Returns (best_a, best_b) seconds."""
    import time as _time
    import jax
    sh_a, ops_a = _bench_setup(inputs, rep_a)
    sh_b, ops_b = _bench_setup(inputs, rep_b)
    best_a = best_b = float("inf")
    for _ in range(iters):
        t0 = _time.time()
        jax.block_until_ready(sh_a(*ops_a))
        best_a = min(best_a, _time.time() - t0)
        t0 = _time.time()
        jax.block_until_ready(sh_b(*ops_b))
        best_b = min(best_b, _time.time() - t0)
    return best_a, best_b
